# revision 1
# baseline (speedup 1.0000x reference)
"""AdapterFormer block (MHA + 5-branch soft-MoE FFN) on 8 TRN2 NeuronCores.

Data-parallel over batch (dim 1 of x): 64 -> 8 per core, weights
replicated, zero collectives.  Per-core tokens: 1576 = 197*8 (l-major,
tok = l*8 + b), 13 partition-tiles of 128 (last tile 40 valid rows).

Activations alternate between token-major (tok on partitions: LN stats,
softmax routing, per-token combines, residuals) and feature-major (feat
on partitions: matmul contraction layout).  Matmul operands are bf16
(f32 PSUM accumulation); residuals and stats stay f32.  Big weights are
cast f32->bf16 into scratch DRAM once (SWDGE cast-DMA), then streamed
into SBUF transposed via XBAR transpose-DMA chunks.
"""
import os
import sys

import numpy as np

sys.path.insert(0, '/opt/trn_rl_repo')

import concourse.bass as bass
import concourse.mybir as mybir
import concourse.tile as tile
from concourse.bass_utils import run_bass_kernel_spmd
from concourse.masks import make_identity

# Enable the LDWEIGHTS optimizer (hardcoded off in bass_utils): overlaps /
# elides stationary-operand loads, which otherwise cost ~100 ns per matmul.
if os.environ.get('BASS_LDW_OPT', '0') == '1':
    import concourse.bass_utils as _bu
    if not getattr(_bu, '_ldw_patched', False):
        _orig_run_command = _bu.run_command

        def _patched_run_command(cmd, *a, **k):
            cmd = ['--enable-ldw-opt=true' if c == '--enable-ldw-opt=false'
                   else c for c in cmd]
            return _orig_run_command(cmd, *a, **k)

        _bu.run_command = _patched_run_command
        _bu._ldw_patched = True

F32 = mybir.dt.float32
BF16 = mybir.dt.bfloat16
AX = mybir.AxisListType.X
OP = mybir.AluOpType
ACTF = mybir.ActivationFunctionType

L, BT, D = 197, 64, 768
H, HD = 12, 64
E = 4
E1 = E + 1
DF = 4 * D                       # 3072
NCORES = 8
B = BT // NCORES                 # 8
NT = L * B                       # 1576
TT = (NT + 127) // 128           # 13
DSUB = D // 128                  # 6
FSUB = DF // 128                 # 24
QKV3 = 3 * D
LROWS = NT - (TT - 1) * 128      # 40
EPS = 1e-5
SCALE = 0.125                    # 1/sqrt(HD)
NCH = [(0, 512), (512, 512), (1024, 512), (1536, NT - 1536)]  # tok chunks

_DEBUG = bool(int(os.environ.get('BASS_KERNEL_DEBUG', '0')))


def _rows(t):
    return 128 if t < TT - 1 else LROWS


# --------------------------------------------------------------------------
# walrus here rejects instructions with >1 sync-wait entry ("Too many sync
# wait commands"); move extra waits onto single-wait NoOps on the same queue.
_ws_ctr = [0]


def _fix_multiwait(nc, max_waits=1):
    for fn in nc.m.functions:
        for blk in fn.blocks:
            insts = list(blk.instructions)
            out = []
            changed = False
            for inst in insts:
                si = inst.sync_info
                waits = list(si.on_wait) if (si is not None and si.on_wait) else []
                if len(waits) > max_waits:
                    extra, keep = waits[:-max_waits], waits[-max_waits:]
                    for i in range(0, len(extra), max_waits):
                        _ws_ctr[0] += 1
                        out.append(mybir.InstNoOp(
                            name=f"I-ws{_ws_ctr[0]}",
                            sync_info=mybir.SyncInfo(
                                on_wait=list(extra[i:i + max_waits]),
                                on_update=[]),
                            bass_nofuse=True,
                            engine=inst.engine,
                        ))
                    si.on_wait = keep
                    inst.sync_info = si
                    changed = True
                out.append(inst)
            if changed:
                blk.instructions = out
    return nc


# --------------------------------------------------------------------------
def build():
    nc = bass.Bass("TRN2", target_bir_lowering=False, debug=False,
                   num_devices=NCORES)

    x_ext = nc.declare_dram_parameter("x", [L, B, D], F32, isOutput=False)
    w = {}
    for name, shape in [
        ("ln1_g", [D]), ("ln1_b", [D]),
        ("in_proj_w", [QKV3, D]), ("in_proj_b", [QKV3]),
        ("out_proj_w", [D, D]), ("out_proj_b", [D]),
        ("ln2_g", [D]), ("ln2_b", [D]),
        ("c_fc_w", [DF, D]), ("c_fc_b", [DF]),
        ("c_proj_w", [D, DF]), ("c_proj_b", [D]),
        ("eh_w", [E, DF, D]), ("eh_b", [E, DF]),
        ("et_w", [E, D, DF]), ("et_b", [E, D]),
        ("r1_w", [E1, D]), ("r1_b", [E1]),
        ("r2_w", [E1, DF]), ("r2_b", [E1]),
    ]:
        w[name] = nc.declare_dram_parameter(name, shape, F32, isOutput=False)
    out_ext = nc.declare_dram_parameter("out", [L, B, D], F32, isOutput=True)
    out_flat = out_ext[:].rearrange("l b d -> (l b) d")
    x_flat = x_ext[:].rearrange("l b d -> (l b) d")

    dbg = {}
    if _DEBUG:
        for name, shape in [
            ("dbg_ln1fm", [128, DSUB, NT]), ("dbg_qkv5", [128, 3, NT]),
            ("dbg_ofm", [128, DSUB, NT]), ("dbg_x1", [128, TT, D]),
            ("dbg_r1", [128, TT, E1]), ("dbg_oht", [128, FSUB, NT]),
            ("dbg_r2", [128, TT, E1]),
        ]:
            dbg[name] = nc.declare_dram_parameter(name, shape, F32,
                                                  isOutput=True)

    with tile.TileContext(nc) as tc:
        _body(nc, tc, x_flat, w, out_flat, dbg)

    _fix_multiwait(nc)
    return nc


def _body(nc, tc, x_flat, w, out_flat, dbg):
    from contextlib import ExitStack
    with ExitStack() as ctx:
        dram = ctx.enter_context(tc.tile_pool(name="dram", bufs=1,
                                              space="DRAM"))
        big = ctx.enter_context(tc.tile_pool(name="big", bufs=1))
        strm = ctx.enter_context(tc.tile_pool(name="strm", bufs=2))
        const = ctx.enter_context(tc.tile_pool(name="const", bufs=1))
        small = ctx.enter_context(tc.tile_pool(name="small", bufs=2))
        psA = ctx.enter_context(tc.tile_pool(name="psA", bufs=2,
                                             space="PSUM"))
        psB = ctx.enter_context(tc.tile_pool(name="psB", bufs=2,
                                             space="PSUM"))
        psT = ctx.enter_context(tc.tile_pool(name="psT", bufs=2,
                                             space="PSUM"))
        psS = ctx.enter_context(tc.tile_pool(name="psS", bufs=2,
                                             space="PSUM"))

        x1_dram = dram.tile([128, TT, D], F32)

        # ---- constants ----------------------------------------------------
        id_bf = const.tile([128, 128], BF16)
        make_identity(nc, id_bf[:])
        id_f32 = const.tile([128, 128], F32)
        make_identity(nc, id_f32[:])
        ones_bf = const.tile([1, 128], BF16)
        nc.vector.memset(ones_bf[:], 1.0)
        eps_col = const.tile([128, 1], F32)
        nc.vector.memset(eps_col[:], EPS)
        c1702 = const.tile([128, 1], F32)
        nc.vector.memset(c1702[:], 1.702)

        def load_cols(name, n):     # [dim] -> per-partition cols [128, n]
            # contiguous row load (scalar ring) + PE transposes: a scattered
            # per-partition DMA costs ~25 us of 4-byte descriptors on the
            # SWDGE queue and stalls the weight casts behind it.
            row = const.tile([1, DSUB * 128], F32, tag="row_stage")
            nc.scalar.dma_start(row[0:1, 0:n * 128], w[name][:].rearrange(
                "(a d) -> a d", a=1))
            t = const.tile([128, n], F32, tag=f"col_{name}")
            for s in range(n):
                pt = psT.tile([128, 1], F32, tag="tp")
                nc.tensor.transpose(pt[:], row[0:1, s * 128:(s + 1) * 128],
                                    id_f32[0:1, 0:1])
                nc.vector.tensor_copy(t[:, s:s + 1], pt[:])
            return t

        ln1g = load_cols("ln1_g", DSUB)
        ln1b = load_cols("ln1_b", DSUB)
        ln2g = load_cols("ln2_g", DSUB)
        ln2b = load_cols("ln2_b", DSUB)
        opb_row = const.tile([1, D], BF16)
        nc.gpsimd.dma_start(opb_row[:], w["out_proj_b"][:].rearrange("(a d) -> a d", a=1))
        r1b_row = const.tile([1, E1], BF16)
        nc.gpsimd.dma_start(r1b_row[:], w["r1_b"][:].rearrange("(a e) -> a e", a=1))
        r2b_row = const.tile([1, E1], BF16)
        nc.gpsimd.dma_start(r2b_row[:], w["r2_b"][:].rearrange("(a e) -> a e", a=1))
        bh_stack = const.tile([E1, DF], BF16)
        nc.gpsimd.dma_start(bh_stack[0:1, :],
                            w["c_fc_b"][:].rearrange("(a f) -> a f", a=1))
        nc.gpsimd.dma_start(bh_stack[1:, :], w["eh_b"][:])
        bt_stack = const.tile([E1, D], BF16)
        nc.gpsimd.dma_start(bt_stack[0:1, :],
                            w["c_proj_b"][:].rearrange("(a d) -> a d", a=1))
        nc.gpsimd.dma_start(bt_stack[1:, :], w["et_b"][:])

        # routing weights (tiny, cast) -- loaded before the big casts so the
        # gpsimd queue does not stall the PE-side prologue transposes
        r1w_nat = const.tile([E1, D], BF16)
        nc.gpsimd.dma_start(r1w_nat[:], w["r1_w"][:])
        r2w_nat = const.tile([E1, DF], BF16, tag="r2wnat")
        nc.gpsimd.dma_start(r2w_nat[:], w["r2_w"][:])

        # ---- bf16 weight scratch in DRAM (SWDGE cast-DMAs), in use-order -
        wqkv_bf = dram.tile([QKV3, D], BF16)
        nc.gpsimd.dma_start(wqkv_bf[:], w["in_proj_w"][:])
        # in_proj bias cols: scattered 4-byte DMA (~25us of descriptors) --
        # after the qkv cast on the SWDGE queue, before the big head/tail
        # casts; not needed until the QKV PSUM copy-out.
        bqkv = const.tile([128, 18], F32)
        with nc.allow_non_contiguous_dma(reason="tiny per-partition col"):
            nc.gpsimd.dma_start(
                bqkv[:], w["in_proj_b"][:].rearrange("(s p) -> p s", p=128))
        wout_bf = dram.tile([D, D], BF16)
        nc.gpsimd.dma_start(wout_bf[:], w["out_proj_w"][:])
        wh_bf = dram.tile([E1 * DF, D], BF16)
        nc.gpsimd.dma_start(wh_bf[0:DF, :], w["c_fc_w"][:])
        nc.gpsimd.dma_start(
            wh_bf[DF:, :], w["eh_w"][:].rearrange("e f d -> (e f) d"))
        wt_bf = dram.tile([E1 * D, DF], BF16)
        nc.gpsimd.dma_start(wt_bf[0:D, :], w["c_proj_w"][:])
        nc.gpsimd.dma_start(
            wt_bf[D:, :], w["et_w"][:].rearrange("e d f -> (e d) f"))

        r1_tm = const.tile([128, TT, E1], F32)
        r2_tm = const.tile([128, TT, E1], F32)
        r1T = const.tile([E1, NT], BF16)
        r2T = const.tile([E1, NT], BF16)

        # ---- load x token-major ------------------------------------------
        x_tm = big.tile([128, TT, D], F32, tag="xo")
        nc.vector.memset(x_tm[:, TT - 1, :], 0.0)
        for t in range(TT):
            r = _rows(t)
            nc.scalar.dma_start(x_tm[0:r, t, :],
                                x_flat[t * 128: t * 128 + r, :])

        # ---- layernorm helper --------------------------------------------
        lnt = big.tile([128, D], BF16, tag="lnt")

        def layernorm_to_fm(dst_fm, g_cols, b_cols):
            for t in range(TT):
                r = _rows(t)
                xs = x_tm[0:r, t, :]
                s1 = small.tile([128, 1], F32, tag="ln_s1")
                nc.vector.reduce_sum(s1[0:r, :], xs, AX)
                sq = small.tile([128, 1], F32, tag="ln_sq")
                xsq = small.tile([128, D], BF16, tag="ln_xsq")
                nc.scalar.activation(xsq[0:r, :], xs, ACTF.Square,
                                     accum_out=sq[0:r, :])
                mu = small.tile([128, 1], F32, tag="ln_mu")
                nc.vector.tensor_scalar(mu[0:r, :], s1[0:r, :], 1.0 / D,
                                        None, OP.mult)
                mu2 = small.tile([128, 1], F32, tag="ln_mu2")
                nc.vector.tensor_tensor(mu2[0:r, :], mu[0:r, :], mu[0:r, :],
                                        OP.mult)
                var = small.tile([128, 1], F32, tag="ln_var")
                nc.vector.scalar_tensor_tensor(
                    out=var[0:r, :], in0=sq[0:r, :], scalar=1.0 / D,
                    in1=mu2[0:r, :], op0=OP.mult, op1=OP.subtract)
                sd = small.tile([128, 1], F32, tag="ln_sd")
                nc.scalar.activation(sd[0:r, :], var[0:r, :], ACTF.Sqrt,
                                     bias=eps_col[0:r, :])
                a_col = small.tile([128, 1], F32, tag="ln_a")
                nc.vector.reciprocal(a_col[0:r, :], sd[0:r, :])
                b_col = small.tile([128, 1], F32, tag="ln_b")
                nc.vector.scalar_tensor_tensor(
                    out=b_col[0:r, :], in0=mu[0:r, :], scalar=-1.0,
                    in1=a_col[0:r, :], op0=OP.mult, op1=OP.mult)
                nc.scalar.activation(lnt[0:r, :], xs, ACTF.Identity,
                                     bias=b_col[0:r, :], scale=a_col[0:r, :])
                for s in range(DSUB):
                    pt = psT.tile([128, 128], BF16, tag="tp")
                    nc.tensor.transpose(pt[:, 0:r],
                                        lnt[0:r, s * 128:(s + 1) * 128],
                                        id_bf[0:r, 0:r])
                    nc.vector.tensor_scalar(
                        dst_fm[:, s, t * 128:t * 128 + r], pt[:, 0:r],
                        g_cols[:, s:s + 1], b_cols[:, s:s + 1],
                        OP.mult, OP.add)

        # ---- LN1 ----------------------------------------------------------
        ln_fm = big.tile([128, DSUB, NT], BF16, tag="ln")
        layernorm_to_fm(ln_fm, ln1g, ln1b)
        if _DEBUG:
            nc.gpsimd.dma_start(dbg["dbg_ln1fm"][:], ln_fm[:])

        # ---- QKV + attention, interleaved per head-pair tile mt ----------
        o_fm = big.tile([128, DSUB, NT], BF16, tag="oa")
        o_lb = o_fm[:].rearrange("p m (l b) -> p m l b", b=B)
        LT = [(0, 128), (128, L - 128)]
        for mt in range(DSUB):
            qkv5 = big.tile([128, 3, NT], BF16, tag="qk")
            for j, m in enumerate([mt, 6 + mt, 12 + mt]):
                wq = strm.tile([128, DSUB, 128], BF16, tag=f"wq{j}")
                nc.sync.dma_start_transpose(
                    wq[:], wqkv_bf[m * 128:(m + 1) * 128, :])
                for c0, cn in NCH:
                    pa = psA.tile([128, 512], F32, tag="mm")
                    for s in range(DSUB):
                        nc.tensor.matmul(pa[:, 0:cn], wq[:, s, :],
                                         ln_fm[:, s, c0:c0 + cn],
                                         start=(s == 0), stop=(s == DSUB - 1))
                    if j == 0:
                        nc.vector.tensor_scalar(
                            qkv5[:, j, c0:c0 + cn], pa[:, 0:cn],
                            bqkv[:, m:m + 1], SCALE, OP.add, OP.mult)
                    else:
                        nc.vector.tensor_scalar(
                            qkv5[:, j, c0:c0 + cn], pa[:, 0:cn],
                            bqkv[:, m:m + 1], None, OP.add)
            if _DEBUG and mt == 0:
                nc.gpsimd.dma_start(dbg["dbg_qkv5"][:], qkv5[:])
            qkv_lb = qkv5[:].rearrange("p j (l b) -> p j l b", b=B)
            v_all = small.tile([128, B, 2, 128], BF16, tag="v_tm")
            for b in range(B):
                vT2 = qkv_lb[:, 2, :, b]              # [128, 197] both heads
                for jj, (m0, mc) in enumerate(LT):
                    pt = psT.tile([128, 128], BF16, tag="tp")
                    nc.tensor.transpose(pt[0:mc, :], vT2[:, m0:m0 + mc],
                                        id_bf[:])
                    nc.vector.tensor_copy(v_all[0:mc, b, jj, :],
                                          pt[0:mc, :])
            for h in (2 * mt, 2 * mt + 1):
                po = (h % 2) * 64
                for b in range(B):
                    v_tm = v_all[:, b, :, :]
                    qT = qkv_lb[po:po + 64, 0, :, b]
                    kT = qkv_lb[po:po + 64, 1, :, b]
                    vT = qkv_lb[po:po + 64, 2, :, b]
                    attn = small.tile([128, 2, L], BF16, tag="attn")
                    rs = small.tile([128, 2], F32, tag="attn_rs")
                    for i, (l0, lc) in enumerate(LT):
                        ps = psS.tile([128, L], F32, tag="att")
                        nc.tensor.matmul(ps[0:lc, :], qT[:, l0:l0 + lc], kT,
                                         start=True, stop=True)
                        sums = small.tile([128, 1], F32, tag="attn_sum")
                        nc.scalar.activation(attn[0:lc, i, :], ps[0:lc, :],
                                             ACTF.Exp,
                                             accum_out=sums[0:lc, :])
                        nc.vector.reciprocal(rs[0:lc, i:i + 1],
                                             sums[0:lc, :])
                        nc.vector.tensor_scalar(
                            attn[0:lc, i, :], attn[0:lc, i, :],
                            rs[0:lc, i:i + 1], None, OP.mult)
                    attnT = small.tile([128, 2, L], BF16, tag="attnT")
                    for jj, (m0, mc) in enumerate(LT):
                        for i, (l0, lc) in enumerate(LT):
                            pt = psT.tile([128, 128], BF16, tag="tp")
                            nc.tensor.transpose(
                                pt[0:mc, 0:lc], attn[0:lc, i, m0:m0 + mc],
                                id_bf[0:lc, 0:lc])
                            nc.vector.tensor_copy(
                                attnT[0:mc, jj, l0:l0 + lc], pt[0:mc, 0:lc])
                    po_ps = psB.tile([64, L], F32, tag="mm2")
                    for jj, (m0, mc) in enumerate(LT):
                        nc.tensor.matmul(po_ps[:],
                                         v_tm[0:mc, jj, po:po + 64],
                                         attnT[0:mc, jj, :],
                                         start=(jj == 0), stop=(jj == 1))
                    nc.vector.tensor_copy(o_lb[po:po + 64, mt, :, b],
                                          po_ps[:])
        if _DEBUG:
            nc.gpsimd.dma_start(dbg["dbg_ofm"][:], o_fm[:])

        # ---- out_proj (token-major) + residual into x_tm ------------------
        woutT = big.tile([128, DSUB, D], BF16, tag="wr")
        nc.sync.dma_start_transpose(woutT[:], wout_bf[:])
        DCH = [(0, 512), (512, 256)]
        for t in range(TT):
            r = _rows(t)
            for c0, cn in DCH:
                pa = psA.tile([128, 512], F32, tag="mm")
                for s in range(DSUB):
                    nc.tensor.matmul(
                        pa[0:r, 0:cn], o_fm[:, s, t * 128:t * 128 + r],
                        woutT[:, s, c0:c0 + cn],
                        start=(s == 0), stop=False)
                nc.tensor.matmul(pa[0:r, 0:cn], ones_bf[0:1, 0:r],
                                 opb_row[0:1, c0:c0 + cn],
                                 start=False, stop=True)
                nc.vector.tensor_tensor(
                    x_tm[0:r, t, c0:c0 + cn], pa[0:r, 0:cn],
                    x_tm[0:r, t, c0:c0 + cn], OP.add)
        if _DEBUG:
            nc.gpsimd.dma_start(dbg["dbg_x1"][:], x_tm[:])

        # ---- spill x1, LN2 ------------------------------------------------
        nc.scalar.dma_start(x1_dram[:], x_tm[:])
        ln2_fm = big.tile([128, DSUB, NT], BF16, tag="ln")
        layernorm_to_fm(ln2_fm, ln2g, ln2b)

        # routing weights transposed: [dsub*128, 5] via PE transpose
        r1wT = const.tile([128, DSUB, E1], BF16)
        for s in range(DSUB):
            pt = psT.tile([128, E1], BF16, tag="tp")
            nc.tensor.transpose(pt[:], r1w_nat[:, s * 128:(s + 1) * 128],
                                id_bf[0:E1, 0:E1])
            nc.vector.tensor_copy(r1wT[:, s, :], pt[:])
        r2wT = const.tile([128, FSUB, E1], BF16)
        for s in range(FSUB):
            pt = psT.tile([128, E1], BF16, tag="tp")
            nc.tensor.transpose(pt[:], r2w_nat[:, s * 128:(s + 1) * 128],
                                id_bf[0:E1, 0:E1])
            nc.vector.tensor_copy(r2wT[:, s, :], pt[:])

        # ---- routing helper (token-major logits, no max-sub: tiny logits)
        def routing(act_fm, nsub, wT, b_row, r_tm, rT):
            for t in range(TT):
                r = _rows(t)
                pr = psB.tile([128, 512], F32, tag="mm2")
                for s in range(nsub):
                    nc.tensor.matmul(pr[0:r, 0:E1],
                                     act_fm[:, s, t * 128:t * 128 + r],
                                     wT[:, s, :],
                                     start=(s == 0), stop=False)
                nc.tensor.matmul(pr[0:r, 0:E1], ones_bf[0:1, 0:r],
                                 b_row[0:1, :], start=False, stop=True)
                e_t = small.tile([128, E1], F32, tag="rt_exp")
                sums = small.tile([128, 1], F32, tag="rt_sum")
                nc.scalar.activation(e_t[0:r, :], pr[0:r, 0:E1], ACTF.Exp,
                                     accum_out=sums[0:r, :])
                rsum = small.tile([128, 1], F32, tag="rt_rsum")
                nc.vector.reciprocal(rsum[0:r, :], sums[0:r, :])
                nc.vector.tensor_scalar(r_tm[0:r, t, :], e_t[0:r, :],
                                        rsum[0:r, :], None, OP.mult)
                ptb = psT.tile([E1, 128], F32, tag="tp")
                nc.tensor.transpose(ptb[:, 0:r], r_tm[0:r, t, :],
                                    id_f32[0:r, 0:r])
                nc.vector.tensor_copy(rT[:, t * 128:t * 128 + r],
                                      ptb[:, 0:r])

        routing(ln2_fm, DSUB, r1wT, r1b_row, r1_tm, r1T)
        if _DEBUG:
            nc.gpsimd.dma_start(dbg["dbg_r1"][:], r1_tm[:])

        # ---- head stage ---------------------------------------------------
        # oh_s (token-major, bf16 accum) -> quickgelu -> transpose into oht
        oht = big.tile([128, FSUB, NT], BF16, tag="xo")
        oh_s = big.tile([128, TT, 512], BF16, tag="oa")
        for sl in range(DF // 512):
            for t in range(TT):
                r = _rows(t)
                pb = psB.tile([128, 512], F32, tag="mm2")
                nc.tensor.matmul(pb[0:r, :], r1T[:, t * 128:t * 128 + r],
                                 bh_stack[:, sl * 512:(sl + 1) * 512],
                                 start=True, stop=True)
                nc.vector.tensor_copy(oh_s[0:r, t, :], pb[0:r, :])
            for e in range(E1):
                wch = strm.tile([128, DSUB, 512], BF16, tag="s2")
                nc.sync.dma_start_transpose(
                    wch[:],
                    wh_bf[e * DF + sl * 512: e * DF + (sl + 1) * 512, :])
                for t in range(TT):
                    r = _rows(t)
                    pa = psA.tile([128, 512], F32, tag="mm")
                    for s in range(DSUB):
                        nc.tensor.matmul(
                            pa[0:r, :],
                            ln2_fm[:, s, t * 128:t * 128 + r],
                            wch[:, s, :],
                            start=(s == 0), stop=(s == DSUB - 1))
                    nc.vector.scalar_tensor_tensor(
                        out=oh_s[0:r, t, :], in0=pa[0:r, :],
                        scalar=r1_tm[0:r, t, e:e + 1],
                        in1=oh_s[0:r, t, :], op0=OP.mult, op1=OP.add)
            for t in range(TT):
                r = _rows(t)
                sig = small.tile([128, 512], BF16, tag="sig")
                nc.scalar.activation(sig[0:r, :], oh_s[0:r, t, :],
                                     ACTF.Sigmoid, scale=c1702[0:r, :])
                nc.vector.tensor_tensor(oh_s[0:r, t, :], oh_s[0:r, t, :],
                                        sig[0:r, :], OP.mult)
                for j in range(4):
                    pt = psT.tile([128, 128], BF16, tag="tp")
                    nc.tensor.transpose(pt[:, 0:r],
                                        oh_s[0:r, t, j * 128:(j + 1) * 128],
                                        id_bf[0:r, 0:r])
                    nc.vector.tensor_copy(
                        oht[:, sl * 4 + j, t * 128:t * 128 + r], pt[:, 0:r])
        if _DEBUG:
            nc.gpsimd.dma_start(dbg["dbg_oht"][:], oht[:])

        # ---- r2 routing ---------------------------------------------------
        routing(oht, FSUB, r2wT, r2b_row, r2_tm, r2T)
        if _DEBUG:
            nc.gpsimd.dma_start(dbg["dbg_r2"][:], r2_tm[:])

        # ---- tail stage + residual + store -------------------------------
        out_s = big.tile([128, TT, 384], F32, tag="ln")
        for dsl in range(2):
            d0 = dsl * 384
            for t in range(TT):
                r = _rows(t)
                pb = psB.tile([128, 512], F32, tag="mm2")
                nc.tensor.matmul(pb[0:r, 0:384],
                                 r2T[:, t * 128:t * 128 + r],
                                 bt_stack[:, d0:d0 + 384],
                                 start=True, stop=True)
                x1s = small.tile([128, 384], F32, tag="x1s")
                nc.scalar.dma_start(x1s[0:r, :], x1_dram[0:r, t, d0:d0 + 384])
                nc.vector.tensor_tensor(out_s[0:r, t, :], pb[0:r, 0:384],
                                        x1s[0:r, :], OP.add)
            for e in range(E1):
                wch0 = strm.tile([128, 12, 384], BF16, tag="s2")
                nc.sync.dma_start_transpose(
                    wch0[:], wt_bf[e * D + d0: e * D + d0 + 384, 0:12 * 128])
                wch1 = strm.tile([128, 12, 384], BF16, tag="s2")
                nc.sync.dma_start_transpose(
                    wch1[:], wt_bf[e * D + d0: e * D + d0 + 384, 12 * 128:])
                for t in range(TT):
                    r = _rows(t)
                    pa = psA.tile([128, 512], F32, tag="mm")
                    for s in range(FSUB):
                        wc = wch0 if s < 12 else wch1
                        nc.tensor.matmul(
                            pa[0:r, 0:384],
                            oht[:, s, t * 128:t * 128 + r],
                            wc[:, s % 12, :],
                            start=(s == 0), stop=(s == FSUB - 1))
                    nc.vector.scalar_tensor_tensor(
                        out=out_s[0:r, t, :], in0=pa[0:r, 0:384],
                        scalar=r2_tm[0:r, t, e:e + 1],
                        in1=out_s[0:r, t, :], op0=OP.mult, op1=OP.add)
            for t in range(TT):
                r = _rows(t)
                nc.scalar.dma_start(
                    out_flat[t * 128:t * 128 + r, d0:d0 + 384],
                    out_s[0:r, t, :])


# --------------------------------------------------------------------------
_cache = {}


def _get_nc():
    if 'nc' not in _cache:
        _cache['nc'] = build()
    return _cache['nc']


def _run(inputs, trace=False, trace_kwargs=None):
    nc = _get_nc()
    full = {k: np.ascontiguousarray(np.asarray(v), dtype=np.float32)
            for k, v in inputs.items()}
    in_maps = []
    for c in range(NCORES):
        m = {k: v for k, v in full.items() if k != 'x'}
        m['x'] = np.ascontiguousarray(full['x'][:, c * B:(c + 1) * B, :])
        in_maps.append(m)
    res = run_bass_kernel_spmd(nc, in_maps, core_ids=list(range(NCORES)),
                               trace=trace, **(trace_kwargs or {}))
    out = np.concatenate([res.results[c]['out'] for c in range(NCORES)],
                         axis=1)
    return out, res


def kernel(**inputs) -> np.ndarray:
    out, _ = _run(inputs, trace=False)
    return out

